# revision 1
# baseline (speedup 1.0000x reference)
"""Multi-head attention with RoPE on 8 Trainium2 NeuronCores.

Sharding: core i handles batch (i // 4) and heads 4*(i % 4) .. 4*(i % 4)+3
(data parallel on B=2, tensor parallel on the 16 heads / the matching
column blocks of Wq/Wk/Wv and row block of Wo). Each core computes its
partial output projection [S, D] (transposed on device); the host sums
the 4 tensor-parallel partials per batch and adds bo.

Per-core device pipeline (all matmuls float32r):
  1. x [2048,1024] DMA'd natural, PE-transposed to xT [din, s] in SBUF.
  2. qT/kT = Wq/Wk.T @ xT (+bias via ACT), RoPE applied on DVE with
     host-precomputed cos / sign-folded-sin tables in the transposed
     layout; v = xT.T @ Wv natural (+bias via K=1 ones-matmul), stored
     ones-augmented [s, 65] per head.
  3. Per head pair, per 512-wide q tile, per 128-wide k chunk:
     scoresT = kT.T-block @ qT (two heads packed in the 128x128 array via
     row tiling at partitions 0/64, separate PSUM banks), ACT exp with
     scale 1/sqrt(dk), then AV: v_aug.T @ expT accumulates [65, 512] —
     row 64 is the softmax denominator (ones column).
  4. 1/Z via DVE reciprocal, broadcast across 64 partitions with a K=1
     ones-matmul, context normalized on DVE into ctxT (f32r).
  5. out_T = Wo_slice.T @ ctxT -> [1024, 2048] partial, DMA'd out.
"""
import os
import sys

if '/opt/trn_rl_repo' not in sys.path:
    sys.path.insert(0, '/opt/trn_rl_repo')

import ml_dtypes
import numpy as np

import concourse.bass as bass
import concourse.mybir as mybir
from concourse.tile import TileContext
from concourse.bass_utils import run_bass_kernel_spmd

F32 = mybir.dt.float32
F32R = mybir.dt.float32r
BF16 = mybir.dt.bfloat16

B, S, D = 2, 2048, 1024
H, DK = 16, 64
TP = 4                  # tensor-parallel ways (head groups)
HPC = H // TP           # heads per core = 4
DH = HPC * DK           # per-core projection width = 256
NT = 512                # moving-operand tile (f32r max)
SC = S // 128           # 16 s-chunks
KC = D // 128           # 8 contraction chunks over D
MC = DH // 128          # 2 dout chunks per core
NQT = S // NT           # 4 q tiles
SH = 2                  # s-halves for the xT buffer

_ENGINES = {
    mybir.EngineType.PE,
    mybir.EngineType.Activation,
    mybir.EngineType.DVE,
    mybir.EngineType.SP,
    mybir.EngineType.Pool,
}


def _split_multi_waits(nc):
    """This walrus build accepts one sync-wait per engine instruction;
    move Tile's extra waits onto preceding same-engine NoOps."""
    for fn in nc.m.functions:
        for blk in fn.blocks:
            insts = blk.instructions
            i = 0
            while i < len(insts):
                inst = insts[i]
                si = getattr(inst, "sync_info", None)
                if (
                    si is not None
                    and len(si.on_wait) > 1
                    and getattr(inst, "engine", None) in _ENGINES
                ):
                    extra = si.on_wait[:-1]
                    si.on_wait = si.on_wait[-1:]
                    nops = [
                        mybir.InstNoOp(
                            name=nc.get_next_instruction_name(),
                            engine=inst.engine,
                            bass_nofuse=True,
                            sync_info=mybir.SyncInfo(on_wait=[w], on_update=[]),
                        )
                        for w in extra
                    ]
                    insts[i:i] = nops
                    i += len(nops)
                i += 1


def _build_core_body(nc, tens, sb_main, tc):
    """Emit one full attention pass. `tens` maps dram tensor name -> handle."""
    x_d, wq_d, wk_d, wv_d, wo_d = (tens[k] for k in ("x_b", "wq", "wk", "wv", "wo"))
    bq_d, bk_d, bv_d = tens["bq"], tens["bk"], tens["bv"]
    cos_d, sin_d, ones_d = tens["cos_t"], tens["sin_s"], tens["ones"]
    out_d = tens["outT"]
    phases = os.environ.get("KPHASES", "123")

    with (
        tc.tile_pool(name="sb_p1", bufs=1) as sb_p1,
        tc.tile_pool(name="ps", bufs=1, space="PSUM") as psp,
    ):
        # Cross-phase PSUM tags (8 banks):
        #   big  [128,1024]x2 - q/k projection units, then score pairs
        #   ps_v [128,256]x2  - v projection, then output projection
        #   av   [65,512]x2   - attention-value accumulators
        def ps_big(name):
            return psp.tile([128, 2 * NT], F32, tag="big", bufs=2, name=name)

        # ---- constants / weights ----
        ones_f = sb_main.tile([1, 128], F32, tag="ones_f")
        nc.sync.dma_start(ones_f[:], ones_d.ap())
        ones_b = sb_main.tile([1, 128], BF16, tag="ones_b")
        nc.scalar.copy(ones_b[:], ones_f[:])
        ones_r = sb_main.tile([1, 128], F32R, tag="ones_r")
        nc.scalar.copy(ones_r[:], ones_f[:])

        wq_t = sb_main.tile([128, KC, DH], BF16, tag="wq")
        wk_t = sb_main.tile([128, KC, DH], BF16, tag="wk")
        wv_t = sb_main.tile([128, KC, DH], BF16, tag="wv")
        nc.gpsimd.dma_start(wq_t[:], wq_d.ap().rearrange("(c p) m -> p c m", p=128))
        nc.gpsimd.dma_start(wk_t[:], wk_d.ap().rearrange("(c p) m -> p c m", p=128))
        nc.gpsimd.dma_start(wv_t[:], wv_d.ap().rearrange("(c p) m -> p c m", p=128))
        wo_t = sb_main.tile([128, MC, D], F32R, tag="wo")
        nc.gpsimd.dma_start(wo_t[:], wo_d.ap().rearrange("(c p) m -> p c m", p=128))

        bq_t = sb_main.tile([128, MC], F32, tag="bq")
        bk_t = sb_main.tile([128, MC], F32, tag="bk")
        nc.sync.dma_start(bq_t[:], bq_d.ap().rearrange("(c p) -> p c", p=128))
        nc.sync.dma_start(bk_t[:], bk_d.ap().rearrange("(c p) -> p c", p=128))
        bv_t = sb_main.tile([1, DH], BF16, tag="bv")
        nc.gpsimd.dma_start(bv_t[:], bv_d.ap())

        cos_t = sb_p1.tile([128, S], F32, tag="cos")
        sin_t = sb_p1.tile([128, S], F32, tag="sin")
        nc.sync.dma_start(cos_t[:], cos_d.ap())
        nc.sync.dma_start(sin_t[:], sin_d.ap())

        # ---- persistent activations ----
        qT = sb_main.tile([128, MC, S], F32R, tag="qT")
        kT = sb_main.tile([128, MC, S], F32R, tag="kT")
        v_aug = sb_main.tile([128, SC, HPC, DK + 1], F32R, tag="v_aug")
        ctx = sb_main.tile([128, MC, S], F32R, tag="ctx")

        # ones column of v_aug: func(0*in + 1) = 1.0 for every (sc, h)
        with nc.allow_low_precision(reason="f32r ones column"):
            nc.scalar.activation(
                v_aug[:, :, :, DK:DK + 1],
                cos_t[:, 0:SC * HPC].rearrange(
                    "p (a b o) -> p a b o", a=SC, b=HPC),
                mybir.ActivationFunctionType.Identity, bias=1.0, scale=0.0)

        # -------- phase 1: xT via 2-byte DMA transpose, q/k/v, RoPE --------
        for half in range(SH):
            sh = S // SH                      # 1024 s per half
            xt = sb_p1.tile([128, KC, sh], BF16, tag="xt")
            for dc in range(KC):
                nc.sync.dma_start_transpose(
                    xt[:, dc, :],
                    x_d.ap()[half * sh:(half + 1) * sh,
                             dc * 128:(dc + 1) * 128])

            # q/k projections (one [128, 1024] unit each) + RoPE
            for mc in range(MC):
                for w_t, b_t, dstT in ((wq_t, bq_t, qT), (wk_t, bk_t, kT)):
                    s0 = half * sh
                    pqk = ps_big("pqk")
                    for nt in range(sh // NT):
                        for kc in range(KC):
                            nc.tensor.matmul(
                                pqk[:, nt * NT:(nt + 1) * NT],
                                w_t[:, kc, mc * 128:(mc + 1) * 128],
                                xt[:, kc, nt * NT:(nt + 1) * NT],
                                start=(kc == 0), stop=(kc == KC - 1))
                    raw = sb_p1.tile([128, sh], F32, tag="raw", bufs=2)
                    nc.scalar.activation(
                        raw[:], pqk[:],
                        mybir.ActivationFunctionType.Identity,
                        bias=b_t[:, mc:mc + 1], scale=1.0)
                    # RoPE: dst = raw*cos - r2, where r2[p] =
                    # raw[p^32]*sin_s[p^32] = -rot_term[p] (sign-folded
                    # sin_s; input bases aligned).
                    a_t = sb_p1.tile([128, sh], F32, tag="ropeA", bufs=2)
                    r_t = sb_p1.tile([128, sh], F32, tag="ropeR", bufs=2)
                    cs = cos_t[:, s0:s0 + sh]
                    ss = sin_t[:, s0:s0 + sh]
                    nc.vector.tensor_mul(a_t[:], raw[:], cs)
                    for g in range(4):
                        dst_p = g * 32
                        src_p = (g ^ 1) * 32
                        nc.vector.tensor_mul(
                            r_t[dst_p:dst_p + 32, :],
                            raw[src_p:src_p + 32, :],
                            ss[src_p:src_p + 32, :])
                    with nc.allow_low_precision(reason="f32r matmul input"):
                        nc.vector.tensor_sub(
                            dstT[:, mc, s0:s0 + sh], a_t[:], r_t[:])

            # v natural projection, ones-augmented
            for sc in range(SC // SH):
                s0c = half * (SC // SH) + sc
                pv = psp.tile([128, DH], F32, tag="ps_v", bufs=2, name="pv")
                for kc in range(KC):
                    nc.tensor.matmul(
                        pv[:], xt[:, kc, sc * 128:(sc + 1) * 128],
                        wv_t[:, kc, :], start=(kc == 0), stop=False)
                nc.tensor.matmul(
                    pv[:], ones_b[:], bv_t[:], start=False, stop=True)
                nc.scalar.copy(
                    v_aug[:, s0c, :, 0:DK],
                    pv[:].rearrange("p (h d) -> p h d", h=HPC))

        if phases == "1":
            return
        # ------------- phase 2: attention per head pair -------------
        pa_tiles = {}
        for qt in range(NQT):
            for pc in range(MC):        # head pair = (2pc, 2pc+1) local
                q0 = qt * NT
                pa0 = psp.tile([DK + 1, NT], F32, tag="ps_av0", bufs=1,
                               name="pa0")
                pa1 = psp.tile([DK + 1, NT], F32, tag="ps_av1", bufs=1,
                               name="pa1")
                for kc in range(SC):
                    k0 = kc * 128
                    psc = ps_big("psc")
                    nc.tensor.matmul(
                        psc[:, 0:NT], kT[0:64, pc, k0:k0 + 128],
                        qT[0:64, pc, q0:q0 + NT],
                        start=True, stop=True, tile_position=(0, 0))
                    nc.tensor.matmul(
                        psc[:, NT:2 * NT], kT[64:128, pc, k0:k0 + 128],
                        qT[64:128, pc, q0:q0 + NT],
                        start=True, stop=True, tile_position=(64, 0))
                    ep = sb_main.tile([128, 2 * NT], F32R, tag="expp", bufs=3)
                    with nc.allow_low_precision(reason="f32r exp"):
                        nc.scalar.activation(
                            ep[:], psc[:], mybir.ActivationFunctionType.Exp,
                            scale=0.125)
                    nc.tensor.matmul(
                        pa0[:], v_aug[:, kc, 2 * pc, :], ep[:, 0:NT],
                        start=(kc == 0), stop=(kc == SC - 1))
                    nc.tensor.matmul(
                        pa1[:], v_aug[:, kc, 2 * pc + 1, :], ep[:, NT:2 * NT],
                        start=(kc == 0), stop=(kc == SC - 1))
                # one DVE copy per head frees the AV psum slot; the
                # normalize chains are batched after all attention units
                for h, pa in ((0, pa0), (1, pa1)):
                    pa_sb = sb_main.tile(
                        [DK + 1, NT], F32, tag=f"pa_sb{qt}_{pc}_{h}", bufs=1,
                        name=f"pa_sb{qt}_{pc}_{h}")
                    nc.vector.tensor_copy(pa_sb[:], pa[:])
                    pa_tiles[(qt, pc, h)] = pa_sb

        # ---- batched normalize: ctx_h = pa[0:64] / Z (Z in row 64) ----
        # pair h0/h1 so the DVE multiply runs full-width [128, NT]
        for qt in range(NQT):
            for pc in range(MC):
                q0 = qt * NT
                pair = sb_main.tile([128, NT], F32, tag="norm_pair", bufs=2)
                bc = sb_main.tile([128, NT], F32, tag="bc", bufs=2)
                for h in range(2):
                    pa_sb = pa_tiles[(qt, pc, h)]
                    nc.vector.tensor_copy(
                        pair[64 * h:64 * h + 64, :], pa_sb[0:DK, :])
                    rz = sb_main.tile([1, NT], F32R, tag=f"recip{h}", bufs=2)
                    with nc.allow_low_precision(reason="f32r recip"):
                        nc.vector.reciprocal(rz[0:1, :], pa_sb[DK:DK + 1, :])
                    pb = psp.tile([64, NT], F32, tag="ps_v", bufs=2, name="pb")
                    nc.tensor.matmul(pb[:], ones_r[:, 0:64], rz[:],
                                     start=True, stop=True)
                    nc.vector.tensor_copy(bc[64 * h:64 * h + 64, :], pb[:])
                with nc.allow_low_precision(reason="f32r ctx"):
                    nc.vector.tensor_mul(
                        ctx[:, pc, q0:q0 + NT], pair[:], bc[:])

        if phases == "12":
            return
        # ---------------- output projection ----------------
        for st in range(NQT):
            for mc_o in range(D // 128):   # 8 output chunks
                po = psp.tile([128, NT], F32, tag="ps_v", bufs=2, name="po")
                for c in range(MC):
                    nc.tensor.matmul(
                        po[:], wo_t[:, c, mc_o * 128:(mc_o + 1) * 128],
                        ctx[:, c, st * NT:(st + 1) * NT],
                        start=(c == 0), stop=(c == MC - 1))
                osb = sb_main.tile([128, NT], F32, tag="osb", bufs=2)
                nc.vector.tensor_copy(osb[:], po[:])
                nc.sync.dma_start(
                    out_d.ap()[mc_o * 128:(mc_o + 1) * 128,
                               st * NT:(st + 1) * NT],
                    osb[:])


def build_nc(reps=1):
    nc = bass.Bass("TRN2", target_bir_lowering=False, debug=False)
    tens = {
        "x_b": nc.dram_tensor("x_b", [S, D], BF16, kind="ExternalInput"),
        "wq": nc.dram_tensor("wq", [D, DH], F32, kind="ExternalInput"),
        "wk": nc.dram_tensor("wk", [D, DH], F32, kind="ExternalInput"),
        "wv": nc.dram_tensor("wv", [D, DH], F32, kind="ExternalInput"),
        "wo": nc.dram_tensor("wo", [DH, D], F32, kind="ExternalInput"),
        "bq": nc.dram_tensor("bq", [DH], F32, kind="ExternalInput"),
        "bk": nc.dram_tensor("bk", [DH], F32, kind="ExternalInput"),
        "bv": nc.dram_tensor("bv", [1, DH], F32, kind="ExternalInput"),
        "cos_t": nc.dram_tensor("cos_t", [128, S], F32, kind="ExternalInput"),
        "sin_s": nc.dram_tensor("sin_s", [128, S], F32, kind="ExternalInput"),
        "ones": nc.dram_tensor("ones", [1, 128], F32, kind="ExternalInput"),
        "outT": nc.dram_tensor("outT", [D, S], F32, kind="ExternalOutput"),
    }
    with TileContext(nc) as tc:
        with tc.tile_pool(name="sb_main", bufs=1) as sb_main:
            for _ in range(reps):
                _build_core_body(nc, tens, sb_main, tc)
    _split_multi_waits(nc)
    return nc


_NC_CACHE = {}



def _rope_tables():
    inv_freq = 1.0 / (10000.0 ** (np.arange(0, DK, 2, dtype=np.float64) / DK))
    pos = np.arange(S, dtype=np.float64)
    freqs = pos[:, None] * inv_freq[None, :]          # [S, 32]
    p = np.arange(128)
    cos = np.cos(freqs[:, p % 32]).T.astype(np.float32)       # [128, S]
    sgn = np.where((p % 64) < 32, -1.0, 1.0)
    sin = (np.sin(freqs[:, p % 32]) * sgn[None, :]).T.astype(np.float32)
    return np.ascontiguousarray(cos), np.ascontiguousarray(sin)


def kernel(x, Wq, bq, Wk, bk, Wv, bv, Wo, bo, _reps=1):
    x, Wq, bq, Wk, bk = (np.asarray(a, np.float32) for a in (x, Wq, bq, Wk, bk))
    Wv, bv, Wo, bo = (np.asarray(a, np.float32) for a in (Wv, bv, Wo, bo))

    if _reps not in _NC_CACHE:
        _NC_CACHE[_reps] = build_nc(_reps)
    nc = _NC_CACHE[_reps]

    cos_t, sin_s = _rope_tables()
    ones = np.ones((1, 128), np.float32)

    in_maps = []
    for core in range(8):
        b, c = core // TP, core % TP
        sl = slice(c * DH, (c + 1) * DH)
        in_maps.append({
            "x_b": np.ascontiguousarray(x[b]).astype(ml_dtypes.bfloat16),
            "wq": np.ascontiguousarray(Wq[:, sl]),
            "wk": np.ascontiguousarray(Wk[:, sl]),
            "wv": np.ascontiguousarray(Wv[:, sl]),
            "wo": np.ascontiguousarray(Wo[sl, :]),
            "bq": np.ascontiguousarray(bq[sl]),
            "bk": np.ascontiguousarray(bk[sl]),
            "bv": np.ascontiguousarray(bv[sl]).reshape(1, DH),
            "cos_t": cos_t,
            "sin_s": sin_s,
            "ones": ones,
        })

    res = run_bass_kernel_spmd(nc, in_maps, list(range(8)))
    out = np.zeros((B, S, D), np.float32)
    for core in range(8):
        out[core // TP] += res.results[core]["outT"].T
    out += bo[None, None, :]
    return out

